# revision 7
# baseline (speedup 1.0000x reference)
"""Trainium2 Bass kernel for DigitConvolutionalModel (conv3x3 + 4-layer MLP).

Math: out = w4 @ relu(w3 @ relu(w2 @ relu(W1e @ x.T + b1) + b2) + b3) + b4
where W1e folds the 3x3 valid conv into the first dense layer:
    W1e[j, (r+dy)*28 + (c+dx)] += w_conv[dy,dx] * w1[j, r*26 + c]

Sharding: pure data parallel — batch 65536 split as 8192 rows per core,
weights replicated, no collectives (forward only). The x shard is shipped
feature-major ([896(pad), 8192] fp16) so layer-1 rhs tiles DMA directly with
K on partitions; matmuls run fp16 (full PE rate + fast weight load) with fp32
PSUM accumulation and fp32 bias adds.
"""

import numpy as np

import concourse.bacc as bacc
import concourse.bass as bass
import concourse.mybir as mybir
import concourse.tile as tile
from concourse.bass_utils import run_bass_kernel_spmd
from concourse.masks import make_identity

N_CORES = 8
B = 65536
B_LOC = B // N_CORES  # 8192
NT = 512  # batch columns per tile
N_ITERS = B_LOC // NT  # 16
KP = 896  # 784 features zero-padded to 7*128

F32 = mybir.dt.float32
F16 = mybir.dt.float16
RELU = mybir.ActivationFunctionType.Relu


def _build():
    nc = bacc.Bacc("TRN2", target_bir_lowering=False, debug=False)

    x = nc.dram_tensor("x", [KP, B_LOC], F16, kind="ExternalInput")
    w_conv = nc.dram_tensor("w_conv", [3, 3], F32, kind="ExternalInput")
    w1 = nc.dram_tensor("w1", [128, 676], F32, kind="ExternalInput")
    b1 = nc.dram_tensor("b1", [128], F32, kind="ExternalInput")
    w2 = nc.dram_tensor("w2", [512, 128], F32, kind="ExternalInput")
    b2 = nc.dram_tensor("b2", [512], F32, kind="ExternalInput")
    w3 = nc.dram_tensor("w3", [256, 512], F32, kind="ExternalInput")
    b3 = nc.dram_tensor("b3", [256], F32, kind="ExternalInput")
    w4 = nc.dram_tensor("w4", [10, 256], F32, kind="ExternalInput")
    b4 = nc.dram_tensor("b4", [10], F32, kind="ExternalInput")
    out = nc.dram_tensor("out", [B_LOC, 10], F32, kind="ExternalOutput")

    with tile.TileContext(nc) as tc:
        with (
            tc.tile_pool(name="const", bufs=1) as cpool,
            tc.tile_pool(name="wts", bufs=1) as wpool,
            tc.tile_pool(name="wtmp", bufs=2) as wtmp,
            tc.tile_pool(name="xt", bufs=3) as xtpool,
            tc.tile_pool(name="acts", bufs=3) as apool,
            tc.tile_pool(name="outs", bufs=2) as opool,
            tc.tile_pool(name="pmm", bufs=6, space=bass.MemorySpace.PSUM) as pmm,
            tc.tile_pool(name="ps", bufs=2, space=bass.MemorySpace.PSUM) as ps,
        ):
            # ---------------- one-time setup ----------------
            ident = cpool.tile([128, 128], F32)
            make_identity(nc, ident)

            # critical-path first: w_conv + w1 feed the W1e fold that gates L1
            wcsb = cpool.tile([1, 9], F32)
            nc.sync.dma_start(wcsb, w_conv[:].rearrange("a b -> (a b)").unsqueeze(0))
            w1sb = wpool.tile([128, 676], F32)
            nc.sync.dma_start(w1sb, w1[:])
            with nc.allow_non_contiguous_dma(reason="one-time small bias loads"):
                b1sb = cpool.tile([128, 1], F32)
                nc.scalar.dma_start(b1sb, b1[:].unsqueeze(1))
                b2sb = cpool.tile([128, 4], F32)
                nc.scalar.dma_start(b2sb, b2[:].rearrange("(s p) -> p s", p=128))
                b3sb = cpool.tile([128, 2], F32)
                nc.scalar.dma_start(b3sb, b3[:].rearrange("(s p) -> p s", p=128))
                b4sb = cpool.tile([1, 10], F32)
                nc.scalar.dma_start(b4sb, b4[:].unsqueeze(0))

            # broadcast w_conv / b4 across partitions on the idle gpsimd
            wcb = cpool.tile([128, 9], F32)
            nc.gpsimd.partition_broadcast(wcb, wcsb)
            b4bc = cpool.tile([128, 10], F32)
            nc.gpsimd.partition_broadcast(b4bc, b4sb)

            # fold conv into layer-1 weights: W1e [128(j), 896(p)] (zero pad).
            # Taps split 6 on DVE / 3 on gpsimd into two accumulators.
            w1e = wpool.tile([128, KP], F32)
            nc.vector.memset(w1e, 0.0)
            w1e3 = w1e[:, :784].rearrange("p (r c) -> p r c", c=28)
            w13 = w1sb.rearrange("p (r c) -> p r c", c=26)
            for t in range(9):
                dy, dx = t // 3, t % 3
                sl = w1e3[:, dy : dy + 26, dx : dx + 26]
                nc.vector.scalar_tensor_tensor(
                    sl,
                    w13,
                    wcb[:, t : t + 1],
                    sl,
                    mybir.AluOpType.mult,
                    mybir.AluOpType.add,
                )

            # transpose weights into lhsT form (K on partitions), cast fp16
            w1eT = wpool.tile([128, 7, 128], F16)
            for ks in range(7):
                ptw = pmm.tile([128, NT], F32, tag="pmm")
                nc.tensor.transpose(
                    ptw[:, :128], w1e[:, 128 * ks : 128 * (ks + 1)], ident
                )
                nc.vector.tensor_copy(w1eT[:, ks, :], ptw[:, :128])

            w2sb = wpool.tile([128, 4, 128], F32)
            nc.sync.dma_start(w2sb, w2[:].rearrange("(ms mi) k -> mi ms k", mi=128))
            w2T = wpool.tile([128, 512], F16)
            ptw = pmm.tile([128, NT], F32, tag="pmm")
            for ms in range(4):
                nc.tensor.transpose(
                    ptw[:, 128 * ms : 128 * (ms + 1)], w2sb[:, ms, :], ident
                )
            nc.vector.tensor_copy(w2T, ptw)

            w3sb = wpool.tile([128, 2, 512], F32)
            nc.sync.dma_start(w3sb, w3[:].rearrange("(ms mi) k -> mi ms k", mi=128))
            w3T = wpool.tile([128, 4, 256], F16)
            for ks in range(4):
                ptw = pmm.tile([128, NT], F32, tag="pmm")
                for ms in range(2):
                    nc.tensor.transpose(
                        ptw[:, 128 * ms : 128 * (ms + 1)],
                        w3sb[:, ms, 128 * ks : 128 * (ks + 1)],
                        ident,
                    )
                nc.vector.tensor_copy(w3T[:, ks, :], ptw[:, :256])

            w4sb = wpool.tile([10, 256], F32)
            nc.sync.dma_start(w4sb, w4[:])
            w4T = wpool.tile([128, 2, 16], F16)
            for ks in range(2):
                ptw = pmm.tile([128, NT], F32, tag="pmm")
                nc.tensor.transpose(
                    ptw[:, :10],
                    w4sb[:, 128 * ks : 128 * (ks + 1)],
                    ident[:10, :10],
                )
                nc.vector.tensor_copy(w4T[:, ks, :10], ptw[:, :10])

            # ---------------- main loop (software pipelined) ----------------
            x7 = x[:].rearrange("(ks ki) n -> ki ks n", ki=128)

            def emit_l1(i):
                # feature-major x tile comes straight from DRAM
                xT = xtpool.tile([128, 7, NT], F16, tag="xT")
                nc.sync.dma_start(xT, x7[:, :, NT * i : NT * (i + 1)])
                p1 = pmm.tile([128, NT], F32, tag="pmm")
                for ks in range(7):
                    nc.tensor.matmul(
                        p1,
                        w1eT[:, ks, :],
                        xT[:, ks, :],
                        start=(ks == 0),
                        stop=(ks == 6),
                    )
                return p1

            def emit_l4(h3, i):
                # L4 (batch-major out): [128(b), 10] = h3_bs.T @ w4T ; +b4 on DVE
                osb = opool.tile([128, 4, 16], F32, tag="osb")
                for bs in range(4):
                    p4 = ps.tile([128, 16], F32, tag="p4")
                    for ks in range(2):
                        nc.tensor.matmul(
                            p4[:, :10],
                            h3[:, ks, 128 * bs : 128 * (bs + 1)],
                            w4T[:, ks, :10],
                            start=(ks == 0),
                            stop=(ks == 1),
                        )
                    nc.vector.tensor_tensor(
                        osb[:, bs, :10], p4[:, :10], b4bc, mybir.AluOpType.add
                    )
                nc.sync.dma_start(
                    out[NT * i : NT * (i + 1), :].rearrange(
                        "(bs bi) j -> bi bs j", bi=128
                    ),
                    osb[:, :, :10],
                )

            p1_cur = emit_l1(0)
            pend_h3 = None
            for i in range(N_ITERS):
                # L1 relu on ACT while PE streams next iteration's L1
                h1 = apool.tile([128, NT], F16, tag="h1")
                nc.scalar.activation(h1, p1_cur, RELU, bias=b1sb)
                if i + 1 < N_ITERS:
                    p1_cur = emit_l1(i + 1)

                # L2: [512, NT] = w2 @ h1 ; relu+bias on DVE
                h2 = apool.tile([128, 4, NT], F16, tag="h2")
                for ms in range(4):
                    p2 = pmm.tile([128, NT], F32, tag="pmm")
                    nc.tensor.matmul(
                        p2,
                        w2T[:, 128 * ms : 128 * (ms + 1)],
                        h1,
                        start=True,
                        stop=True,
                    )
                    nc.vector.tensor_scalar(
                        h2[:, ms, :],
                        p2,
                        b2sb[:, ms : ms + 1],
                        0.0,
                        mybir.AluOpType.add,
                        mybir.AluOpType.max,
                    )

                # previous iteration's tiny L4 matmuls fill the h2 relu wait
                if pend_h3 is not None:
                    emit_l4(pend_h3, i - 1)

                # L3: [256, NT] = w3 @ h2 ; relu on ACT
                h3 = apool.tile([128, 2, NT], F16, tag="h3")
                for ms in range(2):
                    p3 = pmm.tile([128, NT], F32, tag="pmm")
                    for ks in range(4):
                        nc.tensor.matmul(
                            p3,
                            w3T[:, ks, 128 * ms : 128 * (ms + 1)],
                            h2[:, ks, :],
                            start=(ks == 0),
                            stop=(ks == 3),
                        )
                    nc.scalar.activation(
                        h3[:, ms, :], p3, RELU, bias=b3sb[:, ms : ms + 1]
                    )
                pend_h3 = h3

            emit_l4(pend_h3, N_ITERS - 1)

    nc.compile()
    return nc


_NC = None


def _get_nc():
    global _NC
    if _NC is None:
        _NC = _build()
    return _NC


def _prep_x(x):
    """Shard batch across cores; cast fp16, zero-pad features to 896 and
    transpose to feature-major [896, B_LOC] per core."""
    xs = np.asarray(x, dtype=np.float16).reshape(N_CORES, B_LOC, 784)
    xp = np.zeros((N_CORES, KP, B_LOC), np.float16)
    xp[:, :784, :] = xs.transpose(0, 2, 1)
    return xp


def _run(inputs, **kw):
    nc = _get_nc()
    xp = _prep_x(inputs["x"])
    weights = {
        k: np.ascontiguousarray(inputs[k], dtype=np.float32)
        for k in ("w_conv", "w1", "b1", "w2", "b2", "w3", "b3", "w4", "b4")
    }
    in_maps = [{"x": xp[c], **weights} for c in range(N_CORES)]
    res = run_bass_kernel_spmd(nc, in_maps, core_ids=list(range(N_CORES)), **kw)
    full = np.concatenate([res.results[c]["out"] for c in range(N_CORES)], axis=0)
    return full, res


def kernel(**inputs) -> np.ndarray:
    full, _ = _run(inputs)
    return full


# revision 8
# speedup vs baseline: 1.0262x; 1.0262x over previous
"""Trainium2 Bass kernel for DigitConvolutionalModel (conv3x3 + 4-layer MLP).

Math: out = w4 @ relu(w3 @ relu(w2 @ relu(W1e @ x.T + b1) + b2) + b3) + b4
where W1e folds the 3x3 valid conv into the first dense layer:
    W1e[j, (r+dy)*28 + (c+dx)] += w_conv[dy,dx] * w1[j, r*26 + c]

Sharding: pure data parallel — batch 65536 split as 8192 rows per core,
weights replicated, no collectives (forward only). The x shard is shipped
feature-major ([896(pad), 8192] fp16) so layer-1 rhs tiles DMA directly with
K on partitions; the output is returned feature-major ([10, 8192] fp32) and
transposed during the host-side gather. Matmuls run fp16 (full PE rate) with
fp32 PSUM accumulation and fp32 bias adds.
"""

import numpy as np

import concourse.bacc as bacc
import concourse.bass as bass
import concourse.mybir as mybir
import concourse.tile as tile
from concourse.bass_utils import run_bass_kernel_spmd
from concourse.masks import make_identity

N_CORES = 8
B = 65536
B_LOC = B // N_CORES  # 8192
NT = 512  # batch columns per tile
N_ITERS = B_LOC // NT  # 16
KP = 896  # 784 features zero-padded to 7*128

F32 = mybir.dt.float32
F16 = mybir.dt.float16
RELU = mybir.ActivationFunctionType.Relu
ADD = mybir.AluOpType.add
MULT = mybir.AluOpType.mult
MAX = mybir.AluOpType.max


def _build():
    nc = bacc.Bacc("TRN2", target_bir_lowering=False, debug=False)

    x = nc.dram_tensor("x", [KP, B_LOC], F16, kind="ExternalInput")
    w_conv = nc.dram_tensor("w_conv", [3, 3], F32, kind="ExternalInput")
    w1 = nc.dram_tensor("w1", [128, 676], F32, kind="ExternalInput")
    b1 = nc.dram_tensor("b1", [128], F32, kind="ExternalInput")
    w2 = nc.dram_tensor("w2", [512, 128], F32, kind="ExternalInput")
    b2 = nc.dram_tensor("b2", [512], F32, kind="ExternalInput")
    w3 = nc.dram_tensor("w3", [256, 512], F32, kind="ExternalInput")
    b3 = nc.dram_tensor("b3", [256], F32, kind="ExternalInput")
    w4 = nc.dram_tensor("w4", [10, 256], F32, kind="ExternalInput")
    b4 = nc.dram_tensor("b4", [10], F32, kind="ExternalInput")
    out = nc.dram_tensor("out", [10, B_LOC], F32, kind="ExternalOutput")

    with tile.TileContext(nc) as tc:
        with (
            tc.tile_pool(name="const", bufs=1) as cpool,
            tc.tile_pool(name="wts", bufs=1) as wpool,
            tc.tile_pool(name="xt", bufs=3) as xtpool,
            tc.tile_pool(name="acts", bufs=3) as apool,
            tc.tile_pool(name="outs", bufs=2) as opool,
            tc.tile_pool(name="pmm", bufs=6, space=bass.MemorySpace.PSUM) as pmm,
            tc.tile_pool(name="ps", bufs=2, space=bass.MemorySpace.PSUM) as ps,
        ):
            # ---------------- one-time setup ----------------
            # critical-path first: w_conv + w1 feed the W1e fold that gates L1
            wcsb = cpool.tile([1, 9], F32)
            nc.sync.dma_start(wcsb, w_conv[:].rearrange("a b -> (a b)").unsqueeze(0))
            w1sb = wpool.tile([128, 676], F32)
            nc.sync.dma_start(w1sb, w1[:])

            ident = cpool.tile([128, 128], F32)
            make_identity(nc, ident)
            ones1 = cpool.tile([1, 128], F32)
            nc.gpsimd.memset(ones1, 1.0)

            with nc.allow_non_contiguous_dma(reason="one-time small bias loads"):
                b1sb = cpool.tile([128, 1], F32)
                nc.scalar.dma_start(b1sb, b1[:].unsqueeze(1))
                b2sb = cpool.tile([128, 4], F32)
                nc.scalar.dma_start(b2sb, b2[:].rearrange("(s p) -> p s", p=128))
                b3sb = cpool.tile([128, 2], F32)
                nc.scalar.dma_start(b3sb, b3[:].rearrange("(s p) -> p s", p=128))
                b4sb = cpool.tile([10, 1], F32)
                nc.scalar.dma_start(b4sb, b4[:].unsqueeze(1))

            # broadcast w_conv across partitions: ones[1,128].T @ wc[1,9]
            pwc = ps.tile([128, 512], F32, tag="ptw")
            nc.tensor.matmul(pwc[:, :9], ones1, wcsb, start=True, stop=True)
            wcb = cpool.tile([128, 9], F32)
            nc.vector.tensor_copy(wcb, pwc[:, :9])

            # weight transposes that don't depend on the W1e fold go first
            # (PE warms up while DVE runs the fold taps)
            w2sb = wpool.tile([128, 4, 128], F32)
            nc.sync.dma_start(w2sb, w2[:].rearrange("(ms mi) k -> mi ms k", mi=128))
            w2T = wpool.tile([128, 512], F16)
            ptw = ps.tile([128, 512], F32, tag="ptw")
            for ms in range(4):
                nc.tensor.transpose(
                    ptw[:, 128 * ms : 128 * (ms + 1)], w2sb[:, ms, :], ident
                )
            nc.vector.tensor_copy(w2T, ptw)

            w3sb = wpool.tile([128, 2, 512], F32)
            nc.sync.dma_start(w3sb, w3[:].rearrange("(ms mi) k -> mi ms k", mi=128))
            w3T = wpool.tile([128, 4, 256], F16)
            for ks in range(4):
                ptw = ps.tile([128, 512], F32, tag="ptw")
                for ms in range(2):
                    nc.tensor.transpose(
                        ptw[:, 128 * ms : 128 * (ms + 1)],
                        w3sb[:, ms, 128 * ks : 128 * (ks + 1)],
                        ident,
                    )
                nc.vector.tensor_copy(w3T[:, ks, :], ptw[:, :256])

            # w4T [128(k), 2(ks), 10(j)]: stationary lhsT for feature-major L4
            w4sb = wpool.tile([10, 256], F32)
            nc.sync.dma_start(w4sb, w4[:])
            w4T = wpool.tile([128, 2, 16], F16)
            for ks in range(2):
                ptw = ps.tile([128, 512], F32, tag="ptw")
                nc.tensor.transpose(
                    ptw[:, :10],
                    w4sb[:, 128 * ks : 128 * (ks + 1)],
                    ident[:10, :10],
                )
                nc.vector.tensor_copy(w4T[:, ks, :10], ptw[:, :10])

            # fold conv into layer-1 weights: W1e [128(j), 896(p)] (zero pad)
            w1e = wpool.tile([128, KP], F32)
            nc.vector.memset(w1e, 0.0)
            w1e3 = w1e[:, :784].rearrange("p (r c) -> p r c", c=28)
            w13 = w1sb.rearrange("p (r c) -> p r c", c=26)
            for t in range(9):
                dy, dx = t // 3, t % 3
                sl = w1e3[:, dy : dy + 26, dx : dx + 26]
                nc.vector.scalar_tensor_tensor(
                    sl, w13, wcb[:, t : t + 1], sl, MULT, ADD
                )

            w1eT = wpool.tile([128, 7, 128], F16)
            for ks in range(7):
                ptw = ps.tile([128, 512], F32, tag="ptw")
                nc.tensor.transpose(
                    ptw[:, :128], w1e[:, 128 * ks : 128 * (ks + 1)], ident
                )
                nc.vector.tensor_copy(w1eT[:, ks, :], ptw[:, :128])

            # ---------------- main loop ----------------
            x7 = x[:].rearrange("(ks ki) n -> ki ks n", ki=128)
            for i in range(N_ITERS):
                # feature-major x tile comes straight from DRAM
                xT = xtpool.tile([128, 7, NT], F16, tag="xT")
                nc.sync.dma_start(xT, x7[:, :, NT * i : NT * (i + 1)])

                # L1: [128, NT] = W1e @ xT ; relu on ACT
                p1 = pmm.tile([128, NT], F32, tag="pmm")
                for ks in range(7):
                    nc.tensor.matmul(
                        p1,
                        w1eT[:, ks, :],
                        xT[:, ks, :],
                        start=(ks == 0),
                        stop=(ks == 6),
                    )
                h1 = apool.tile([128, NT], F16, tag="h1")
                nc.scalar.activation(h1, p1, RELU, bias=b1sb)

                # L2: [512, NT] = w2 @ h1 ; relu+bias on DVE (last tile on ACT)
                h2 = apool.tile([128, 4, NT], F16, tag="h2")
                for ms in range(4):
                    p2 = pmm.tile([128, NT], F32, tag="pmm")
                    nc.tensor.matmul(
                        p2,
                        w2T[:, 128 * ms : 128 * (ms + 1)],
                        h1,
                        start=True,
                        stop=True,
                    )
                    if ms == 3:
                        nc.scalar.activation(
                            h2[:, ms, :], p2, RELU, bias=b2sb[:, ms : ms + 1]
                        )
                    else:
                        nc.vector.tensor_scalar(
                            h2[:, ms, :], p2, b2sb[:, ms : ms + 1], 0.0, ADD, MAX
                        )

                # L3: [256, NT] = w3 @ h2 ; relu on ACT
                h3 = apool.tile([128, 2, NT], F16, tag="h3")
                for ms in range(2):
                    p3 = pmm.tile([128, NT], F32, tag="pmm")
                    for ks in range(4):
                        nc.tensor.matmul(
                            p3,
                            w3T[:, ks, 128 * ms : 128 * (ms + 1)],
                            h2[:, ks, :],
                            start=(ks == 0),
                            stop=(ks == 3),
                        )
                    nc.scalar.activation(
                        h3[:, ms, :], p3, RELU, bias=b3sb[:, ms : ms + 1]
                    )

                # L4 feature-major: [10, NT] = w4 @ h3 ; +b4 on DVE
                p4 = ps.tile([128, 512], F32, tag="ptw")
                for ks in range(2):
                    nc.tensor.matmul(
                        p4[:10, :],
                        w4T[:, ks, :10],
                        h3[:, ks, :],
                        start=(ks == 0),
                        stop=(ks == 1),
                    )
                osb = opool.tile([10, NT], F32, tag="osb")
                nc.vector.tensor_scalar_add(osb, p4[:10, :], b4sb)
                nc.sync.dma_start(out[:, NT * i : NT * (i + 1)], osb)

    nc.compile()
    return nc


_NC = None


def _get_nc():
    global _NC
    if _NC is None:
        _NC = _build()
    return _NC


def _prep_x(x):
    """Shard batch across cores; cast fp16, zero-pad features to 896 and
    transpose to feature-major [896, B_LOC] per core."""
    xs = np.asarray(x, dtype=np.float16).reshape(N_CORES, B_LOC, 784)
    xp = np.zeros((N_CORES, KP, B_LOC), np.float16)
    xp[:, :784, :] = xs.transpose(0, 2, 1)
    return xp


def _run(inputs, **kw):
    nc = _get_nc()
    xp = _prep_x(inputs["x"])
    weights = {
        k: np.ascontiguousarray(inputs[k], dtype=np.float32)
        for k in ("w_conv", "w1", "b1", "w2", "b2", "w3", "b3", "w4", "b4")
    }
    in_maps = [{"x": xp[c], **weights} for c in range(N_CORES)]
    res = run_bass_kernel_spmd(nc, in_maps, core_ids=list(range(N_CORES)), **kw)
    # gather/unshard: outputs are feature-major [10, B_LOC] per core
    full = np.concatenate(
        [res.results[c]["out"].T for c in range(N_CORES)], axis=0
    )
    return np.ascontiguousarray(full), res


def kernel(**inputs) -> np.ndarray:
    full, _ = _run(inputs)
    return full


# revision 9
# speedup vs baseline: 1.1013x; 1.0732x over previous
"""Trainium2 Bass kernel for DigitConvolutionalModel (conv3x3 + 4-layer MLP).

Math: out = w4 @ relu(w3 @ relu(w2 @ relu(W1e @ x.T + b1) + b2) + b3) + b4
where W1e folds the 3x3 valid conv into the first dense layer:
    W1e[j, (r+dy)*28 + (c+dx)] += w_conv[dy,dx] * w1[j, r*26 + c]

Sharding: pure data parallel — batch 65536 split as 8192 rows per core,
weights replicated, no collectives (forward only). The x shard is shipped
feature-major ([896(pad), 8192] fp16) so layer-1 rhs tiles DMA directly with
K on partitions; the output is returned feature-major ([10, 8192] fp32) and
transposed during the host-side gather. Matmuls run fp16 (full PE rate) with
fp32 PSUM accumulation and fp32 bias adds.
"""

import numpy as np

import concourse.bacc as bacc
import concourse.bass as bass
import concourse.mybir as mybir
import concourse.tile as tile
from concourse.bass_utils import run_bass_kernel_spmd
from concourse.masks import make_identity

N_CORES = 8
B = 65536
B_LOC = B // N_CORES  # 8192
NT = 512  # batch columns per tile
N_ITERS = B_LOC // NT  # 16
KP = 896  # 784 features zero-padded to 7*128

F32 = mybir.dt.float32
F16 = mybir.dt.float16
RELU = mybir.ActivationFunctionType.Relu
ADD = mybir.AluOpType.add
MULT = mybir.AluOpType.mult
MAX = mybir.AluOpType.max


def _build():
    nc = bacc.Bacc("TRN2", target_bir_lowering=False, debug=False)

    x = nc.dram_tensor("x", [KP, B_LOC], F16, kind="ExternalInput")
    w_conv = nc.dram_tensor("w_conv", [3, 3], F32, kind="ExternalInput")
    w1 = nc.dram_tensor("w1", [128, 676], F32, kind="ExternalInput")
    b1 = nc.dram_tensor("b1", [128], F32, kind="ExternalInput")
    w2 = nc.dram_tensor("w2", [512, 128], F32, kind="ExternalInput")
    b2 = nc.dram_tensor("b2", [512], F32, kind="ExternalInput")
    w3 = nc.dram_tensor("w3", [256, 512], F32, kind="ExternalInput")
    b3 = nc.dram_tensor("b3", [256], F32, kind="ExternalInput")
    w4 = nc.dram_tensor("w4", [10, 256], F32, kind="ExternalInput")
    b4 = nc.dram_tensor("b4", [10], F32, kind="ExternalInput")
    out = nc.dram_tensor("out", [10, B_LOC], F32, kind="ExternalOutput")

    with tile.TileContext(nc) as tc:
        with (
            tc.tile_pool(name="const", bufs=1) as cpool,
            tc.tile_pool(name="wts", bufs=1) as wpool,
            tc.tile_pool(name="xt", bufs=3) as xtpool,
            tc.tile_pool(name="acts", bufs=3) as apool,
            tc.tile_pool(name="outs", bufs=2) as opool,
            tc.tile_pool(name="pmm", bufs=6, space=bass.MemorySpace.PSUM) as pmm,
            tc.tile_pool(name="ps", bufs=2, space=bass.MemorySpace.PSUM) as ps,
        ):
            # ---------------- one-time setup ----------------
            # critical-path first: w_conv + w1 feed the W1e fold that gates L1
            wcsb = cpool.tile([1, 9], F32)
            nc.sync.dma_start(wcsb, w_conv[:].rearrange("a b -> (a b)").unsqueeze(0))
            w1sb = wpool.tile([128, 676], F32)
            nc.sync.dma_start(w1sb, w1[:])

            ident = cpool.tile([128, 128], F32)
            make_identity(nc, ident)
            ones1 = cpool.tile([1, 128], F32)
            nc.gpsimd.memset(ones1, 1.0)

            with nc.allow_non_contiguous_dma(reason="one-time small bias loads"):
                b1sb = cpool.tile([128, 1], F32)
                nc.scalar.dma_start(b1sb, b1[:].unsqueeze(1))
                b2sb = cpool.tile([128, 4], F32)
                nc.scalar.dma_start(b2sb, b2[:].rearrange("(s p) -> p s", p=128))
                b3sb = cpool.tile([128, 2], F32)
                nc.scalar.dma_start(b3sb, b3[:].rearrange("(s p) -> p s", p=128))
                b4sb = cpool.tile([10, 1], F32)
                nc.scalar.dma_start(b4sb, b4[:].unsqueeze(1))

            # broadcast w_conv across partitions: ones[1,128].T @ wc[1,9]
            pwc = ps.tile([128, 512], F32, tag="ptw")
            nc.tensor.matmul(pwc[:, :9], ones1, wcsb, start=True, stop=True)
            wcb = cpool.tile([128, 9], F32)
            nc.vector.tensor_copy(wcb, pwc[:, :9])

            # fold conv into layer-1 weights: W1e [128(j), 896(p)] (zero pad)
            w1e = wpool.tile([128, KP], F32)
            nc.vector.memset(w1e, 0.0)
            w1e3 = w1e[:, :784].rearrange("p (r c) -> p r c", c=28)
            w13 = w1sb.rearrange("p (r c) -> p r c", c=26)
            for t in range(9):
                dy, dx = t // 3, t % 3
                sl = w1e3[:, dy : dy + 26, dx : dx + 26]
                nc.vector.scalar_tensor_tensor(
                    sl, w13, wcb[:, t : t + 1], sl, MULT, ADD
                )

            # weight transposes that don't depend on the W1e fold go first
            # (PE warms up while DVE runs the fold taps)
            w2sb = wpool.tile([128, 4, 128], F32)
            nc.sync.dma_start(w2sb, w2[:].rearrange("(ms mi) k -> mi ms k", mi=128))
            w2T = wpool.tile([128, 512], F16)
            ptw = ps.tile([128, 512], F32, tag="ptw")
            for ms in range(4):
                nc.tensor.transpose(
                    ptw[:, 128 * ms : 128 * (ms + 1)], w2sb[:, ms, :], ident
                )
            nc.vector.tensor_copy(w2T, ptw)

            w3sb = wpool.tile([128, 2, 512], F32)
            nc.sync.dma_start(w3sb, w3[:].rearrange("(ms mi) k -> mi ms k", mi=128))
            w3T = wpool.tile([128, 4, 256], F16)
            for ks in range(4):
                ptw = ps.tile([128, 512], F32, tag="ptw")
                for ms in range(2):
                    nc.tensor.transpose(
                        ptw[:, 128 * ms : 128 * (ms + 1)],
                        w3sb[:, ms, 128 * ks : 128 * (ks + 1)],
                        ident,
                    )
                nc.vector.tensor_copy(w3T[:, ks, :], ptw[:, :256])

            # w4T [128(k), 2(ks), 10(j)]: stationary lhsT for feature-major L4
            w4sb = wpool.tile([10, 256], F32)
            nc.sync.dma_start(w4sb, w4[:])
            w4T = wpool.tile([128, 2, 16], F16)
            for ks in range(2):
                ptw = ps.tile([128, 512], F32, tag="ptw")
                nc.tensor.transpose(
                    ptw[:, :10],
                    w4sb[:, 128 * ks : 128 * (ks + 1)],
                    ident[:10, :10],
                )
                nc.vector.tensor_copy(w4T[:, ks, :10], ptw[:, :10])

            w1eT = wpool.tile([128, 7, 128], F16)
            for ks in range(7):
                ptw = ps.tile([128, 512], F32, tag="ptw")
                nc.tensor.transpose(
                    ptw[:, :128], w1e[:, 128 * ks : 128 * (ks + 1)], ident
                )
                nc.vector.tensor_copy(w1eT[:, ks, :], ptw[:, :128])

            # ---------------- main loop ----------------
            x7 = x[:].rearrange("(ks ki) n -> ki ks n", ki=128)
            pend_h3 = None
            for i in range(N_ITERS):
                # feature-major x tile comes straight from DRAM
                xT = xtpool.tile([128, 7, NT], F16, tag="xT")
                nc.sync.dma_start(xT, x7[:, :, NT * i : NT * (i + 1)])

                # L1: [128, NT] = W1e @ xT ; relu on ACT
                p1 = pmm.tile([128, NT], F32, tag="pmm")
                for ks in range(7):
                    nc.tensor.matmul(
                        p1,
                        w1eT[:, ks, :],
                        xT[:, ks, :],
                        start=(ks == 0),
                        stop=(ks == 6),
                    )
                h1 = apool.tile([128, NT], F16, tag="h1")
                nc.scalar.activation(h1, p1, RELU, bias=b1sb)

                # previous iteration's L4 fills the h1-relu wait on PE
                if pend_h3 is not None:
                    p4 = ps.tile([128, 512], F32, tag="ptw")
                    for ks in range(2):
                        nc.tensor.matmul(
                            p4[:10, :],
                            w4T[:, ks, :10],
                            pend_h3[:, ks, :],
                            start=(ks == 0),
                            stop=(ks == 1),
                        )
                    osb = opool.tile([10, NT], F32, tag="osb")
                    nc.vector.tensor_scalar_add(osb, p4[:10, :], b4sb)
                    nc.sync.dma_start(out[:, NT * (i - 1) : NT * i], osb)

                # L2: [512, NT] = w2 @ h1 ; relu+bias on DVE (last tile on ACT)
                h2 = apool.tile([128, 4, NT], F16, tag="h2")
                for ms in range(4):
                    p2 = pmm.tile([128, NT], F32, tag="pmm")
                    nc.tensor.matmul(
                        p2,
                        w2T[:, 128 * ms : 128 * (ms + 1)],
                        h1,
                        start=True,
                        stop=True,
                    )
                    if ms == 3:
                        nc.scalar.activation(
                            h2[:, ms, :], p2, RELU, bias=b2sb[:, ms : ms + 1]
                        )
                    else:
                        nc.vector.tensor_scalar(
                            h2[:, ms, :], p2, b2sb[:, ms : ms + 1], 0.0, ADD, MAX
                        )

                # L3: [256, NT] = w3 @ h2 ; relu on ACT
                h3 = apool.tile([128, 2, NT], F16, tag="h3")
                for ms in range(2):
                    p3 = pmm.tile([128, NT], F32, tag="pmm")
                    for ks in range(4):
                        nc.tensor.matmul(
                            p3,
                            w3T[:, ks, 128 * ms : 128 * (ms + 1)],
                            h2[:, ks, :],
                            start=(ks == 0),
                            stop=(ks == 3),
                        )
                    nc.scalar.activation(
                        h3[:, ms, :], p3, RELU, bias=b3sb[:, ms : ms + 1]
                    )

                pend_h3 = h3

            # drain the last iteration's L4
            p4 = ps.tile([128, 512], F32, tag="ptw")
            for ks in range(2):
                nc.tensor.matmul(
                    p4[:10, :],
                    w4T[:, ks, :10],
                    pend_h3[:, ks, :],
                    start=(ks == 0),
                    stop=(ks == 1),
                )
            osb = opool.tile([10, NT], F32, tag="osb")
            nc.vector.tensor_scalar_add(osb, p4[:10, :], b4sb)
            nc.sync.dma_start(out[:, NT * (N_ITERS - 1) :], osb)

    nc.compile()
    return nc


_NC = None


def _get_nc():
    global _NC
    if _NC is None:
        _NC = _build()
    return _NC


def _prep_x(x):
    """Shard batch across cores; cast fp16, zero-pad features to 896 and
    transpose to feature-major [896, B_LOC] per core."""
    xs = np.asarray(x, dtype=np.float16).reshape(N_CORES, B_LOC, 784)
    xp = np.zeros((N_CORES, KP, B_LOC), np.float16)
    xp[:, :784, :] = xs.transpose(0, 2, 1)
    return xp


def _run(inputs, **kw):
    nc = _get_nc()
    xp = _prep_x(inputs["x"])
    weights = {
        k: np.ascontiguousarray(inputs[k], dtype=np.float32)
        for k in ("w_conv", "w1", "b1", "w2", "b2", "w3", "b3", "w4", "b4")
    }
    in_maps = [{"x": xp[c], **weights} for c in range(N_CORES)]
    res = run_bass_kernel_spmd(nc, in_maps, core_ids=list(range(N_CORES)), **kw)
    # gather/unshard: outputs are feature-major [10, B_LOC] per core
    full = np.concatenate(
        [res.results[c]["out"].T for c in range(N_CORES)], axis=0
    )
    return np.ascontiguousarray(full), res


def kernel(**inputs) -> np.ndarray:
    full, _ = _run(inputs)
    return full


# revision 10
# speedup vs baseline: 1.1616x; 1.0548x over previous
"""Trainium2 Bass kernel for DigitConvolutionalModel (conv3x3 + 4-layer MLP).

Math: out = w4 @ relu(w3 @ relu(w2 @ relu(W1e @ x.T + b1) + b2) + b3) + b4
where W1e folds the 3x3 valid conv into the first dense layer:
    W1e[j, (r+dy)*28 + (c+dx)] += w_conv[dy,dx] * w1[j, r*26 + c]

Sharding: pure data parallel — batch 65536 split as 8192 rows per core,
weights replicated, no collectives (forward only). The x shard is shipped
feature-major ([896(pad), 8192] fp16) so layer-1 rhs tiles DMA directly with
K on partitions; the output is returned feature-major ([10, 8192] fp32) and
transposed during the host-side gather. Matmuls run fp16 (full PE rate) with
fp32 PSUM accumulation and fp32 bias adds.
"""

import numpy as np

import concourse.bacc as bacc
import concourse.bass as bass
import concourse.mybir as mybir
import concourse.tile as tile
from concourse.bass_utils import run_bass_kernel_spmd
from concourse.masks import make_identity

N_CORES = 8
B = 65536
B_LOC = B // N_CORES  # 8192
NT = 512  # batch columns per tile
N_ITERS = B_LOC // NT  # 16
KP = 896  # 784 features zero-padded to 7*128

F32 = mybir.dt.float32
F16 = mybir.dt.float16
RELU = mybir.ActivationFunctionType.Relu
ADD = mybir.AluOpType.add
MULT = mybir.AluOpType.mult
MAX = mybir.AluOpType.max


def _build():
    nc = bacc.Bacc("TRN2", target_bir_lowering=False, debug=False)

    x = nc.dram_tensor("x", [KP, B_LOC], F16, kind="ExternalInput")
    w_conv = nc.dram_tensor("w_conv", [3, 3], F32, kind="ExternalInput")
    w1 = nc.dram_tensor("w1", [128, 676], F32, kind="ExternalInput")
    b1 = nc.dram_tensor("b1", [128], F32, kind="ExternalInput")
    w2 = nc.dram_tensor("w2", [512, 128], F32, kind="ExternalInput")
    b2 = nc.dram_tensor("b2", [512], F32, kind="ExternalInput")
    w3 = nc.dram_tensor("w3", [256, 512], F32, kind="ExternalInput")
    b3 = nc.dram_tensor("b3", [256], F32, kind="ExternalInput")
    w4 = nc.dram_tensor("w4", [10, 256], F32, kind="ExternalInput")
    b4 = nc.dram_tensor("b4", [10], F32, kind="ExternalInput")
    out = nc.dram_tensor("out", [10, B_LOC], F32, kind="ExternalOutput")

    with tile.TileContext(nc) as tc:
        with (
            tc.tile_pool(name="const", bufs=1) as cpool,
            tc.tile_pool(name="wts", bufs=1) as wpool,
            tc.tile_pool(name="xt", bufs=3) as xtpool,
            tc.tile_pool(name="acts", bufs=3) as apool,
            tc.tile_pool(name="outs", bufs=2) as opool,
            tc.tile_pool(name="pp1", bufs=2, space=bass.MemorySpace.PSUM) as pp1,
            tc.tile_pool(name="pmm", bufs=4, space=bass.MemorySpace.PSUM) as pmm,
            tc.tile_pool(name="ps", bufs=2, space=bass.MemorySpace.PSUM) as ps,
        ):
            # ---------------- one-time setup ----------------
            # critical-path first: w_conv + w1 feed the W1e fold that gates L1
            wcsb = cpool.tile([1, 9], F32)
            nc.sync.dma_start(wcsb, w_conv[:].rearrange("a b -> (a b)").unsqueeze(0))
            w1sb = wpool.tile([128, 676], F32)
            nc.sync.dma_start(w1sb, w1[:])

            ident = cpool.tile([128, 128], F32)
            make_identity(nc, ident)
            ones1 = cpool.tile([1, 128], F32)
            nc.gpsimd.memset(ones1, 1.0)

            with nc.allow_non_contiguous_dma(reason="one-time small bias loads"):
                b1sb = cpool.tile([128, 1], F32)
                nc.scalar.dma_start(b1sb, b1[:].unsqueeze(1))
                b2sb = cpool.tile([128, 4], F32)
                nc.scalar.dma_start(b2sb, b2[:].rearrange("(s p) -> p s", p=128))
                b3sb = cpool.tile([128, 2], F32)
                nc.scalar.dma_start(b3sb, b3[:].rearrange("(s p) -> p s", p=128))
                b4sb = cpool.tile([10, 1], F32)
                nc.scalar.dma_start(b4sb, b4[:].unsqueeze(1))

            # broadcast w_conv across partitions: ones[1,128].T @ wc[1,9]
            pwc = ps.tile([128, 512], F32, tag="ptw")
            nc.tensor.matmul(pwc[:, :9], ones1, wcsb, start=True, stop=True)
            wcb = cpool.tile([128, 9], F32)
            nc.vector.tensor_copy(wcb, pwc[:, :9])

            # fold conv into layer-1 weights: W1e [128(j), 896(p)] (zero pad)
            w1e = wpool.tile([128, KP], F32)
            nc.vector.memset(w1e, 0.0)
            w1e3 = w1e[:, :784].rearrange("p (r c) -> p r c", c=28)
            w13 = w1sb.rearrange("p (r c) -> p r c", c=26)
            for t in range(9):
                dy, dx = t // 3, t % 3
                sl = w1e3[:, dy : dy + 26, dx : dx + 26]
                nc.vector.scalar_tensor_tensor(
                    sl, w13, wcb[:, t : t + 1], sl, MULT, ADD
                )

            # weight transposes that don't depend on the W1e fold go first
            # (PE warms up while DVE runs the fold taps)
            w2sb = wpool.tile([128, 4, 128], F32)
            nc.gpsimd.dma_start(w2sb, w2[:].rearrange("(ms mi) k -> mi ms k", mi=128))
            w2T = wpool.tile([128, 512], F16)
            ptw = ps.tile([128, 512], F32, tag="ptw")
            for ms in range(4):
                nc.tensor.transpose(
                    ptw[:, 128 * ms : 128 * (ms + 1)], w2sb[:, ms, :], ident
                )
            nc.vector.tensor_copy(w2T, ptw)

            w3sb = wpool.tile([128, 2, 512], F32)
            nc.scalar.dma_start(w3sb, w3[:].rearrange("(ms mi) k -> mi ms k", mi=128))
            w3T = wpool.tile([128, 4, 256], F16)
            for ks in range(4):
                ptw = ps.tile([128, 512], F32, tag="ptw")
                for ms in range(2):
                    nc.tensor.transpose(
                        ptw[:, 128 * ms : 128 * (ms + 1)],
                        w3sb[:, ms, 128 * ks : 128 * (ks + 1)],
                        ident,
                    )
                nc.vector.tensor_copy(w3T[:, ks, :], ptw[:, :256])

            # w4T [128(k), 2(ks), 10(j)]: stationary lhsT for feature-major L4
            w4sb = wpool.tile([10, 256], F32)
            nc.gpsimd.dma_start(w4sb, w4[:])
            w4T = wpool.tile([128, 2, 16], F16)
            for ks in range(2):
                ptw = ps.tile([128, 512], F32, tag="ptw")
                nc.tensor.transpose(
                    ptw[:, :10],
                    w4sb[:, 128 * ks : 128 * (ks + 1)],
                    ident[:10, :10],
                )
                nc.vector.tensor_copy(w4T[:, ks, :10], ptw[:, :10])

            w1eT = wpool.tile([128, 7, 128], F16)
            for ks in range(7):
                ptw = ps.tile([128, 512], F32, tag="ptw")
                nc.tensor.transpose(
                    ptw[:, :128], w1e[:, 128 * ks : 128 * (ks + 1)], ident
                )
                nc.vector.tensor_copy(w1eT[:, ks, :], ptw[:, :128])

            # ---------------- main loop ----------------
            # L1 runs one iteration ahead: its 7 matmuls fill the PE wait for
            # the DVE/ACT relu drains of the current iteration.
            x7 = x[:].rearrange("(ks ki) n -> ki ks n", ki=128)

            def emit_l1(i):
                xT = xtpool.tile([128, 7, NT], F16, tag="xT")
                nc.sync.dma_start(xT, x7[:, :, NT * i : NT * (i + 1)])
                p1 = pp1.tile([128, NT], F32, tag="p1")
                for ks in range(7):
                    nc.tensor.matmul(
                        p1,
                        w1eT[:, ks, :],
                        xT[:, ks, :],
                        start=(ks == 0),
                        stop=(ks == 6),
                    )
                return p1

            def emit_l4(h3, i):
                # L4 feature-major: [10, NT] = w4 @ h3 ; +b4 on DVE
                p4 = ps.tile([128, 512], F32, tag="ptw")
                for ks in range(2):
                    nc.tensor.matmul(
                        p4[:10, :],
                        w4T[:, ks, :10],
                        h3[:, ks, :],
                        start=(ks == 0),
                        stop=(ks == 1),
                    )
                osb = opool.tile([10, NT], F32, tag="osb")
                nc.vector.tensor_scalar_add(osb, p4[:10, :], b4sb)
                nc.scalar.dma_start(out[:, NT * i : NT * (i + 1)], osb)

            p1_cur = emit_l1(0)
            pend_h3 = None
            for i in range(N_ITERS):
                h1 = apool.tile([128, NT], F16, tag="h1")
                nc.scalar.activation(h1, p1_cur, RELU, bias=b1sb)

                # previous iteration's L4 fills the h1-relu wait on PE
                if pend_h3 is not None:
                    emit_l4(pend_h3, i - 1)

                # L2: [512, NT] = w2 @ h1 ; relu+bias on DVE (last tile on ACT)
                h2 = apool.tile([128, 4, NT], F16, tag="h2")
                for ms in range(4):
                    p2 = pmm.tile([128, NT], F32, tag="pmm")
                    nc.tensor.matmul(
                        p2,
                        w2T[:, 128 * ms : 128 * (ms + 1)],
                        h1,
                        start=True,
                        stop=True,
                    )
                    if ms == 3:
                        nc.scalar.activation(
                            h2[:, ms, :], p2, RELU, bias=b2sb[:, ms : ms + 1]
                        )
                    else:
                        nc.vector.tensor_scalar(
                            h2[:, ms, :], p2, b2sb[:, ms : ms + 1], 0.0, ADD, MAX
                        )

                # next iteration's L1 fills the h2-relu wait on PE
                if i + 1 < N_ITERS:
                    p1_cur = emit_l1(i + 1)

                # L3: [256, NT] = w3 @ h2 ; relu on ACT
                h3 = apool.tile([128, 2, NT], F16, tag="h3")
                for ms in range(2):
                    p3 = pmm.tile([128, NT], F32, tag="pmm")
                    for ks in range(4):
                        nc.tensor.matmul(
                            p3,
                            w3T[:, ks, 128 * ms : 128 * (ms + 1)],
                            h2[:, ks, :],
                            start=(ks == 0),
                            stop=(ks == 3),
                        )
                    nc.scalar.activation(
                        h3[:, ms, :], p3, RELU, bias=b3sb[:, ms : ms + 1]
                    )
                pend_h3 = h3

            emit_l4(pend_h3, N_ITERS - 1)

    nc.compile()
    return nc


_NC = None


def _get_nc():
    global _NC
    if _NC is None:
        _NC = _build()
    return _NC


def _prep_x(x):
    """Shard batch across cores; cast fp16, zero-pad features to 896 and
    transpose to feature-major [896, B_LOC] per core."""
    xs = np.asarray(x, dtype=np.float16).reshape(N_CORES, B_LOC, 784)
    xp = np.zeros((N_CORES, KP, B_LOC), np.float16)
    xp[:, :784, :] = xs.transpose(0, 2, 1)
    return xp


def _run(inputs, **kw):
    nc = _get_nc()
    xp = _prep_x(inputs["x"])
    weights = {
        k: np.ascontiguousarray(inputs[k], dtype=np.float32)
        for k in ("w_conv", "w1", "b1", "w2", "b2", "w3", "b3", "w4", "b4")
    }
    in_maps = [{"x": xp[c], **weights} for c in range(N_CORES)]
    res = run_bass_kernel_spmd(nc, in_maps, core_ids=list(range(N_CORES)), **kw)
    # gather/unshard: outputs are feature-major [10, B_LOC] per core
    full = np.concatenate(
        [res.results[c]["out"].T for c in range(N_CORES)], axis=0
    )
    return np.ascontiguousarray(full), res


def kernel(**inputs) -> np.ndarray:
    full, _ = _run(inputs)
    return full
